# revision 3
# baseline (speedup 1.0000x reference)
"""Node2Node supervised-contrastive loss on 8 Trainium2 NeuronCores — v2.

Same routing strategy as the baseline (pairs routed to the core owning the
sampled row; anchors permuted per-core into 32 slot-blocks of 128 with
uniform block widths M[j]), but:
  - the gathered row is 256 B (features only, bf16) instead of 512 B:
    positive/valid masks are integer bookkeeping, computed on host and
    streamed as bf16 planes; per-anchor positive counts also host-side.
  - the gather table is an ExternalInput-region DRAM buffer written by the
    kernel (gathers from pool/output regions measured 1.6-5x slower).
  - single_packet=False (single_packet=True crashes the ucode at any size).
  - label-gather machinery (yw/wa/off one-hot select) removed entirely.
  - repeat wraps ALL phases for honest slope timing.

Measured roofline (real HW, 8 cores, distinct index streams): random
dma_gather costs ~9.3 ns/index independent of 256B/512B rows — descriptor-
latency-bound (~150 ns/desc/engine x 16 engines). Per core ~267K pair
descriptors -> ~2.5 ms; all DVE/Act/PE work (~140 us) hides under it.
Alternatives measured and rejected: ap_gather 89 ns/idx; SBUF-source
transpose gather 27 ns/idx (and wrong); indirect_dma_start, 4 swdge
queues, single_packet=True, >=16K-idx calls all crash; 2 swdge queues run
but give no speedup (DMA engines already saturated).
"""

import os
import sys

import numpy as np
import ml_dtypes

sys.path.insert(0, "/opt/trn_rl_repo")

import concourse.bass as bass
import concourse.bacc as bacc
import concourse.mybir as mybir
import concourse.tile as tile
from concourse import bass_utils

F32 = mybir.dt.float32
BF16 = mybir.dt.bfloat16
I16 = mybir.dt.int16
MUL = mybir.AluOpType.mult
ADD = mybir.AluOpType.add
SUB = mybir.AluOpType.subtract
EQ = mybir.AluOpType.is_equal
AFT = mybir.ActivationFunctionType


class CFG:
    def __init__(self, N=100000, D=128, A=4096, S=512, NC=8, TEMP=0.1):
        self.N, self.D, self.A, self.S, self.NC, self.TEMP = N, D, A, S, NC, TEMP
        self.SL = N // NC
        self.NB = A // 128
        self.G = -(-self.SL // 128)
        self.SLP = self.G * 128


REAL = CFG()


def prep(cfg, x, y, anchors, sampled):
    N, A, S, NC, SL, NB = cfg.N, cfg.A, cfg.S, cfg.NC, cfg.SL, cfg.NB
    x = np.ascontiguousarray(np.asarray(x, dtype=np.float32))
    y64 = np.asarray(y, dtype=np.int64)
    anchors = np.asarray(anchors, dtype=np.int64)
    sampled = np.asarray(sampled, dtype=np.int64)

    core_of = sampled // SL
    cnt = np.zeros((A, NC), dtype=np.int64)
    for c in range(NC):
        cnt[:, c] = (core_of == c).sum(1)

    perms, ranks = [], []
    Ms = np.zeros((NC, NB), dtype=np.int64)
    for c in range(NC):
        p = np.argsort(cnt[:, c], kind="stable")
        r = np.empty(A, dtype=np.int64)
        r[p] = np.arange(A)
        perms.append(p)
        ranks.append(r)
        Ms[c] = cnt[p, c].reshape(NB, 128).max(1)
    M = Ms.max(0)
    Cj = np.concatenate([[0], np.cumsum(M)])
    MTOT = int(Cj[-1])

    pos_full = (y64[sampled] == y64[anchors][:, None])      # [A, S] bool

    cores = []
    for c in range(NC):
        perm, rank = perms[c], ranks[c]
        a_list, s_list = np.nonzero(core_of == c)
        local = (sampled[a_list, s_list] - c * SL).astype(np.int64)
        n = cnt[:, c]
        start = np.concatenate([[0], np.cumsum(n)])
        k = np.arange(len(a_list)) - start[a_list]
        r = rank[a_list]
        j, p = r // 128, r % 128
        col = Cj[j] + k
        idxmat = np.zeros((128, MTOT), dtype=np.int16)
        vmask = np.zeros((128, MTOT), dtype=ml_dtypes.bfloat16)
        pmask = np.zeros((128, MTOT), dtype=ml_dtypes.bfloat16)
        idxmat[p, col] = local.astype(np.int16)
        vmask[p, col] = 1.0
        pmask[p, col] = pos_full[a_list, s_list].astype(ml_dtypes.bfloat16)
        flat = idxmat.T.reshape(-1)                          # [MTOT*128]
        L = flat.size // 16
        wrapped = np.zeros((128, L), dtype=np.int16)
        w16 = flat.reshape(L, 16).T
        for g in range(8):
            wrapped[g * 16:(g + 1) * 16, :] = w16

        aperm = anchors[perm]
        xa = x[aperm].reshape(NB, 128, cfg.D).transpose(1, 0, 2).copy()

        xs = np.ones((cfg.SLP, cfg.D), dtype=np.float32)
        xs[:SL] = x[c * SL:(c + 1) * SL]

        cores.append(dict(xs=xs, xa=xa, sidx=wrapped, vmask=vmask, pmask=pmask,
                          tbuf=np.zeros((cfg.SLP, cfg.D),
                                        dtype=ml_dtypes.bfloat16)))

    # global per-anchor positive counts, anchor order (for kernel 2)
    cnt_pos = pos_full.sum(1).astype(np.float32)             # [A]
    return cores, perms, M.astype(int).tolist(), cnt_pos


def build_k1(cfg, M, repeat=1):
    NB, D, G, SLP = cfg.NB, cfg.D, cfg.G, cfg.SLP
    MTOT = sum(M)
    nc = bacc.Bacc("TRN2", target_bir_lowering=False, debug=False,
                   num_devices=cfg.NC)
    xs = nc.dram_tensor("xs", [SLP, D], F32, kind="ExternalInput").ap()
    xa = nc.dram_tensor("xa", [128, NB, D], F32, kind="ExternalInput").ap()
    sidx = nc.dram_tensor("sidx", [128, MTOT * 8], I16, kind="ExternalInput").ap()
    vmask = nc.dram_tensor("vmask", [128, MTOT], BF16, kind="ExternalInput").ap()
    pmask = nc.dram_tensor("pmask", [128, MTOT], BF16, kind="ExternalInput").ap()
    acc_out = nc.dram_tensor("acc", [128, NB, 2], F32, kind="ExternalOutput").ap()
    table = nc.dram_tensor("tbuf", [SLP, D], BF16, kind="ExternalInput").ap()

    with tile.TileContext(nc) as tc:
        if True:
            for _rep in range(repeat):
                # ---- phase A: normalized bf16 feature table ----
                with tc.tile_pool(name="pa", bufs=2) as pa:
                    half = (G + 1) // 2
                    xsr = xs.rearrange("(g p) d -> p g d", p=128)
                    tbr = table.rearrange("(g p) d -> p g d", p=128)
                    for h in range(2):
                        g0 = h * half
                        g1 = min(G, g0 + half)
                        gw = g1 - g0
                        if gw <= 0:
                            continue
                        xt = pa.tile([128, half, D], F32, tag="xt")
                        nc.sync.dma_start(xt[:, :gw, :], xsr[:, g0:g1, :])
                        sq = pa.tile([128, half, D], F32, tag="sq")
                        nc.vector.tensor_tensor(out=sq[:, :gw, :],
                                                in0=xt[:, :gw, :],
                                                in1=xt[:, :gw, :], op=MUL)
                        ss = pa.tile([128, half], F32, tag="ss")
                        nc.vector.reduce_sum(out=ss[:, :gw], in_=sq[:, :gw, :],
                                             axis=mybir.AxisListType.X)
                        nc.scalar.activation(ss[:, :gw], ss[:, :gw], AFT.Sqrt)
                        inv = pa.tile([128, half], F32, tag="inv")
                        nc.vector.reciprocal(inv[:, :gw], ss[:, :gw])
                        tb = pa.tile([128, half, D], BF16, tag="tb")
                        nc.vector.tensor_tensor(
                            out=tb[:, :gw, :], in0=xt[:, :gw, :],
                            in1=inv[:, :gw].unsqueeze(2).to_broadcast(
                                [128, gw, D]), op=MUL)
                        nc.sync.dma_start(tbr[:, g0:g1, :], tb[:, :gw, :])

                # ---- phase B: anchor features + resident masks/indices ----
                with tc.tile_pool(name="pb", bufs=1) as pb, \
                     tc.tile_pool(name="res", bufs=1) as res:
                    xat = pb.tile([128, NB, D], F32)
                    nc.sync.dma_start(xat[:], xa[:])
                    sqa = pb.tile([128, NB, D], F32)
                    nc.vector.tensor_tensor(out=sqa[:], in0=xat[:], in1=xat[:],
                                            op=MUL)
                    ssa = pb.tile([128, NB], F32)
                    nc.vector.reduce_sum(out=ssa[:], in_=sqa[:],
                                         axis=mybir.AxisListType.X)
                    nc.scalar.activation(ssa[:], ssa[:], AFT.Sqrt)
                    inva = pb.tile([128, NB], F32)
                    nc.vector.reciprocal(inva[:], ssa[:])
                    af = res.tile([128, NB, D], BF16)
                    nc.vector.tensor_tensor(
                        out=af[:], in0=xat[:],
                        in1=inva[:].unsqueeze(2).to_broadcast([128, NB, D]),
                        op=MUL)
                    vm = res.tile([128, MTOT], BF16)
                    nc.sync.dma_start(vm[:], vmask[:])
                    pm = res.tile([128, MTOT], BF16)
                    nc.sync.dma_start(pm[:], pmask[:])
                    it = res.tile([128, MTOT * 8], I16)
                    nc.sync.dma_start(it[:], sidx[:])
                    acc = res.tile([128, NB, 2], F32)
                    nc.vector.memset(acc[:], 0.0)

                    # ---- phase C: gather + dot + exp + masked sums ----
                    with tc.tile_pool(name="pcb", bufs=2) as pcb, \
                         tc.tile_pool(name="pc", bufs=3) as pc:
                        Cj = 0
                        for j in range(NB):
                            mj = M[j]
                            st = pcb.tile([128, mj, D], BF16, tag=f"st{j % 2}")
                            nc.gpsimd.dma_gather(
                                st[:], table,
                                it[:, Cj * 8:(Cj + mj) * 8],
                                mj * 128, mj * 128, D, single_packet=False)
                            nc.vector.tensor_tensor(
                                out=st[:], in0=st[:],
                                in1=af[:, j:j + 1, :].to_broadcast([128, mj, D]),
                                op=MUL)
                            w = D // 2
                            while w >= 1:
                                nc.vector.tensor_tensor(
                                    out=st[:, :, 0:w], in0=st[:, :, 0:w],
                                    in1=st[:, :, w:2 * w], op=ADD)
                                w //= 2
                            e = pc.tile([128, mj], BF16, tag="e")
                            nc.scalar.activation(e[:], st[:, :, 0], AFT.Exp,
                                                 scale=1.0 / cfg.TEMP)
                            ev = pc.tile([128, mj], BF16, tag="ev")
                            nc.vector.tensor_tensor(out=ev[:], in0=e[:],
                                                    in1=vm[:, Cj:Cj + mj],
                                                    op=MUL)
                            em = pc.tile([128, mj], BF16, tag="em")
                            nc.vector.tensor_tensor(out=em[:], in0=e[:],
                                                    in1=pm[:, Cj:Cj + mj],
                                                    op=MUL)
                            for q, src in ((0, em), (1, ev)):
                                tmp = pc.tile([128, 1], F32, tag=f"tmp{q}")
                                nc.vector.reduce_sum(out=tmp[:], in_=src[:],
                                                     axis=mybir.AxisListType.X)
                                nc.vector.tensor_tensor(
                                    out=acc[:, j, q:q + 1],
                                    in0=acc[:, j, q:q + 1],
                                    in1=tmp[:], op=ADD)
                            Cj += mj
                    nc.sync.dma_start(acc_out[:], acc[:])
    nc.compile()
    return nc


def build_k2(cfg, repeat=1):
    """Combine per-core partials (host-realigned to anchor order), compute
    per-anchor loss and the total. parts: [128, NC, NB, 2]; cnt: [128, NB]."""
    NB, NC = cfg.NB, cfg.NC
    nc = bacc.Bacc("TRN2", target_bir_lowering=False, debug=False, num_devices=1)
    parts = nc.dram_tensor("parts", [128, NC, NB, 2], F32,
                           kind="ExternalInput").ap()
    cnt_in = nc.dram_tensor("cnt", [128, NB], F32, kind="ExternalInput").ap()
    out = nc.dram_tensor("out", [1, 1], F32, kind="ExternalOutput").ap()
    with tile.TileContext(nc) as tc:
        with tc.tile_pool(name="p", bufs=1) as p, \
             tc.tile_pool(name="ps", bufs=1, space="PSUM") as psp:
            t = p.tile([128, NC, NB, 2], F32)
            nc.sync.dma_start(t[:], parts[:])
            cnt = p.tile([128, NB], F32)
            nc.sync.dma_start(cnt[:], cnt_in[:])
            for _rep in range(repeat):
                s2 = p.tile([128, NB, 2], F32, tag="s2")
                tt = t[:].transpose([0, 2, 3, 1])
                nc.vector.reduce_sum(out=s2[:], in_=tt,
                                     axis=mybir.AxisListType.X)
                n_ = s2[:, :, 0]
                d_ = s2[:, :, 1]
                cz = p.tile([128, NB], F32, tag="cz")
                nc.vector.tensor_scalar(out=cz[:], in0=cnt[:], scalar1=0.0,
                                        scalar2=None, op0=EQ)
                n1 = p.tile([128, NB], F32, tag="n1")
                nc.vector.tensor_tensor(out=n1[:], in0=n_, in1=cz[:], op=ADD)
                c1 = p.tile([128, NB], F32, tag="c1")
                nc.vector.tensor_scalar_max(out=c1[:], in0=cnt[:], scalar1=1.0)
                lnn = p.tile([128, NB], F32, tag="lnn")
                nc.scalar.activation(lnn[:], n1[:], AFT.Ln)
                lnd = p.tile([128, NB], F32, tag="lnd")
                nc.scalar.activation(lnd[:], d_, AFT.Ln)
                df = p.tile([128, NB], F32, tag="df")
                nc.vector.tensor_tensor(out=df[:], in0=lnd[:], in1=lnn[:],
                                        op=SUB)
                rc = p.tile([128, NB], F32, tag="rc")
                nc.vector.reciprocal(rc[:], c1[:])
                pa = p.tile([128, NB], F32, tag="pa")
                nc.vector.tensor_tensor(out=pa[:], in0=df[:], in1=rc[:], op=MUL)
                m = p.tile([128, NB], F32, tag="m")
                nc.scalar.activation(m[:], cz[:], AFT.Copy, scale=-1.0, bias=1.0)
                pa2 = p.tile([128, NB], F32, tag="pa2")
                nc.vector.tensor_tensor(out=pa2[:], in0=pa[:], in1=m[:], op=MUL)
                rs = p.tile([128, 1], F32, tag="rs")
                nc.vector.reduce_sum(out=rs[:], in_=pa2[:],
                                     axis=mybir.AxisListType.X)
                ones = p.tile([128, 1], F32, tag="ones")
                nc.vector.memset(ones[:], 1.0)
                accp = psp.tile([1, 1], F32, tag="accp")
                nc.tensor.matmul(out=accp[:], lhsT=rs[:], rhs=ones[:],
                                 start=True, stop=True)
                res = p.tile([1, 1], F32, tag="res")
                nc.vector.tensor_copy(out=res[:], in_=accp[:])
            nc.sync.dma_start(out[:], res[:])
    nc.compile()
    return nc


def _finish(cfg, cores_results, perms, cnt_pos):
    """Host realign of per-core slot-order partials to anchor order."""
    aligned = np.zeros((cfg.NC, cfg.A, 2), dtype=np.float32)
    for c in range(cfg.NC):
        acc = cores_results[c]["acc"]                      # [128, NB, 2]
        acc_t = acc.transpose(1, 0, 2).reshape(cfg.A, 2)
        aligned[c, perms[c]] = acc_t
    parts = aligned.reshape(cfg.NC, cfg.NB, 128, 2).transpose(2, 0, 1, 3).copy()
    cnt = cnt_pos.reshape(cfg.NB, 128).T.copy()
    return parts, cnt


def _run(cfg, x, y, anchors, sampled):
    cores, perms, M, cnt_pos = prep(cfg, x, y, anchors, sampled)
    nc1 = build_k1(cfg, M)
    in_maps = [dict(xs=c["xs"], xa=c["xa"], sidx=c["sidx"], vmask=c["vmask"],
                    pmask=c["pmask"], tbuf=c["tbuf"]) for c in cores]
    r1 = bass_utils.run_bass_kernel_spmd(nc1, in_maps,
                                         core_ids=list(range(cfg.NC)))
    parts, cnt = _finish(cfg, r1.results, perms, cnt_pos)
    nc2 = build_k2(cfg)
    r2 = bass_utils.run_bass_kernel_spmd(nc2, [dict(parts=parts, cnt=cnt)],
                                         core_ids=[0])
    val = np.float32(r2.results[0]["out"].reshape(-1)[0])
    return val


def kernel(x, y, anchors, sampled):
    val = _run(REAL, np.asarray(x), np.asarray(y), np.asarray(anchors),
               np.asarray(sampled))
    return np.asarray(val, dtype=np.float32)
